# revision 6
# baseline (speedup 1.0000x reference)
"""Trainium2 Bass kernel for the DLGN kernel-machine problem (fp8 DoubleRow).

Reference (fp32):
    ig = inp @ g0; dg = data @ g0
    K  = sig(4 ig) @ sig(4 dg).T
    for l in 1..3: ig @= g_l; dg @= g_l; K *= (sig(4 ig) @ sig(4 dg).T)/512
    out = K @ alphas                                  # [4096]

Design (8 cores, R=2 x C=4: inp rows split in 2, data rows in 4; each core
computes y_partial[r-block] over its data block; host sums C partials):
  - Gate chains are LINEAR in the preactivations: ig_l = inp @ (g0 g1..g_l).
    The kernel precomputes both weight-product chains on device (WP_l and
    its transpose chain V_l, plain f32r matmuls) and computes every layer's
    preactivation directly from inp/data -- no serial layer dependency, no
    PSUM->SBUF preactivation copies.
  - K-product runs in fp8 (float8e4) DoubleRow mode at 0.5 cyc/row (4x the
    f32r rate). Precision is recovered by tanh-centering: store
    t = fp8(tanh(2x)) (= 2 sig(4x) - 1), then
      sig-gram G = (T + u_i + u_d + 512) / 4,  T = t_i^T t_d (fp8 gram),
    with u_* = exact column sums of t (ones-matmuls). u_i enters as a
    per-partition scalar in the vector-engine multiply; u_d via one extra
    zero-padded DoubleRow pair (row0 = 64 * fp8(u_d/64)). The constant
    1/(4*2048^3) is applied once to y at the end (2^-35). Measured rel err
    ~1e-2 vs the 2e-2 gate (CPU model 9.8e-3).
  - ACT engine runs ONLY Tanh (scale=2, direct fp8 output -- bit-exact vs
    RNE cast); running K-product stays on DVE scalar_tensor_tensor with the
    alphas folded into layer 0 and accum_out row-sum on layer 3.
"""

import numpy as np
import ml_dtypes

import concourse.tile as tile
from concourse import bacc, mybir
from concourse.bass_utils import run_bass_kernel_spmd

WIDTH = 512
DEPTH = 4
DIM = 512
N_I = 4096
N_D = 8192
R, C = 2, 4
NI_SH = N_I // R  # 2048
ND_SH = N_D // C  # 2048
I_CHUNKS = NI_SH // 128  # 16
DSL = ND_SH // 512  # 4 d-blocks of 512
KC = DIM // 128  # 4

F32 = mybir.dt.float32
F32R = mybir.dt.float32r
FP8 = mybir.dt.float8e4
AFT = mybir.ActivationFunctionType
MULT = mybir.AluOpType.mult
ADD = mybir.AluOpType.add
DR = mybir.MatmulPerfMode.DoubleRow

_NC = None


def _build_fp8(repeat=1):
    nc = bacc.Bacc("TRN2", target_bir_lowering=False, debug=False, num_devices=8)

    inpT_d = nc.dram_tensor("inpT", [DIM, NI_SH], F32R, kind="ExternalInput")
    dataT_d = nc.dram_tensor("dataT", [DIM, ND_SH], F32R, kind="ExternalInput")
    gating_d = nc.dram_tensor("gating", [DEPTH, DIM, DIM], F32R, kind="ExternalInput")
    g0T_d = nc.dram_tensor("g0T", [DIM, DIM], F32R, kind="ExternalInput")
    alphas_d = nc.dram_tensor("alphas_b", [128, ND_SH], F32, kind="ExternalInput")
    ones_d = nc.dram_tensor("ones8", [128, 2, 16], FP8, kind="ExternalInput")
    corrl_d = nc.dram_tensor("corrl8", [128, 2, 128], FP8, kind="ExternalInput")
    y_d = nc.dram_tensor("y", [128, I_CHUNKS], F32, kind="ExternalOutput")

    from contextlib import nullcontext

    with tile.TileContext(nc) as tc:
        with (
            tc.tile_pool(name="wp", bufs=1) as wppool,
            tc.tile_pool(name="io", bufs=1) as iopool,
            tc.tile_pool(name="c8", bufs=1) as cpool,
            tc.For_i(0, repeat, 1) if repeat > 1 else nullcontext(),
        ):
            WP = wppool.tile([128, DEPTH, KC, DIM], F32R)
            inpT_sb = iopool.tile([128, KC, NI_SH], F32R, name="inpT_sb")
            dataT_sb = iopool.tile([128, KC, ND_SH], F32R, name="dataT_sb")
            alp = iopool.tile([128, ND_SH], F32, name="alp")
            ones8 = cpool.tile([128, 2, 16], FP8, name="ones8")
            corrl8 = cpool.tile([128, 2, 128], FP8, name="corrl8")
            nc.sync.dma_start(
                WP[:, 0], gating_d.ap()[0].rearrange("(k p) n -> p k n", p=128)
            )
            nc.sync.dma_start(ones8[:], ones_d.ap())
            nc.sync.dma_start(corrl8[:], corrl_d.ap())
            nc.sync.dma_start(alp[:], alphas_d.ap())
            inpT_r = inpT_d.ap().rearrange("(k p) n -> p k n", p=128)
            dataT_r = dataT_d.ap().rearrange("(k p) n -> p k n", p=128)
            for k in range(KC):
                nc.scalar.dma_start(inpT_sb[:, k], inpT_r[:, k])
                nc.scalar.dma_start(dataT_sb[:, k], dataT_r[:, k])

            # ---- Phase 0: weight-product chains ----
            with (
                tc.tile_pool(name="gv", bufs=1) as gvpool,
                tc.tile_pool(name="vv", bufs=2) as vpool,
                tc.tile_pool(name="p0", bufs=2, space="PSUM") as p0,
            ):
                g_all = gvpool.tile([128, DEPTH, KC, DIM], F32R)
                for l in range(1, DEPTH):
                    nc.sync.dma_start(
                        g_all[:, l],
                        gating_d.ap()[l].rearrange("(k p) n -> p k n", p=128),
                    )
                v_prev = vpool.tile([128, KC, DIM], F32R, tag="v")
                nc.sync.dma_start(
                    v_prev[:], g0T_d.ap().rearrange("(k p) n -> p k n", p=128)
                )
                for l in range(1, DEPTH):
                    # WP_l = WP_{l-1} @ g_l  (lhsT = V_{l-1})
                    for ma in range(KC):
                        ps = p0.tile([128, 512], F32, tag="p0")
                        for k in range(KC):
                            nc.tensor.matmul(
                                ps[:],
                                v_prev[:, k, ma * 128 : (ma + 1) * 128],
                                g_all[:, l, k, :],
                                start=(k == 0),
                                stop=(k == KC - 1),
                            )
                        nc.vector.tensor_copy(WP[:, l, ma, :], ps[:])
                    if l < DEPTH - 1:
                        v_next = vpool.tile([128, KC, DIM], F32R, tag="v", name=f"v{l}")
                        for mw in range(KC):
                            ps = p0.tile([128, 512], F32, tag="p0")
                            for k in range(KC):
                                nc.tensor.matmul(
                                    ps[:],
                                    g_all[:, l, k, mw * 128 : (mw + 1) * 128],
                                    v_prev[:, k, :],
                                    start=(k == 0),
                                    stop=(k == KC - 1),
                                )
                            nc.vector.tensor_copy(v_next[:, mw, :], ps[:])
                        v_prev = v_next

            # ---- Phase A: all gates (i and d side), u-sums ----
            with (
                tc.tile_pool(name="sig", bufs=1) as sigpool,
                tc.tile_pool(name="kb", bufs=3) as kpool,
                tc.tile_pool(name="scr", bufs=4) as scrpool,
                tc.tile_pool(name="gps", bufs=3, space="PSUM") as gpsum,
                tc.tile_pool(name="udp", bufs=2, space="PSUM") as udpsum,
                tc.tile_pool(name="kps", bufs=3, space="PSUM") as kpsum,
            ):
                t8i = sigpool.tile([128, DEPTH, KC, NI_SH], FP8)
                t8d = sigpool.tile([128, DEPTH, KC, ND_SH], FP8)
                corri8 = sigpool.tile([128, 2, DEPTH, NI_SH], FP8)
                corrd8 = sigpool.tile([128, 2, DEPTH, ND_SH], FP8)
                y_acc = sigpool.tile([128, I_CHUNKS], F32)
                nc.gpsimd.memset(corri8[:], 0.0)
                nc.gpsimd.memset(corrd8[:], 0.0)
                for l in range(DEPTH):
                    nc.gpsimd.memset(corri8[0:1, 1, l, :], 1.0)
                    nc.gpsimd.memset(corrd8[0:1, 0, l, :], 1.0)
                nc.gpsimd.memset(y_acc[:], 0.0)

                for l in range(DEPTH):
                    for m in range(KC):
                        for nb in range(NI_SH // 512):
                            sl = slice(nb * 512, (nb + 1) * 512)
                            ps = gpsum.tile([128, 512], F32, tag="gps")
                            for k in range(KC):
                                nc.tensor.matmul(
                                    ps[:],
                                    WP[:, l, k, m * 128 : (m + 1) * 128],
                                    inpT_sb[:, k, sl],
                                    start=(k == 0),
                                    stop=(k == KC - 1),
                                )
                            nc.scalar.activation(
                                t8i[:, l, m, sl], ps[:], AFT.Tanh, scale=2.0
                            )
                    for m in range(KC):
                        for nb in range(ND_SH // 512):
                            sl = slice(nb * 512, (nb + 1) * 512)
                            ps = gpsum.tile([128, 512], F32, tag="gps")
                            for k in range(KC):
                                nc.tensor.matmul(
                                    ps[:],
                                    WP[:, l, k, m * 128 : (m + 1) * 128],
                                    dataT_sb[:, k, sl],
                                    start=(k == 0),
                                    stop=(k == KC - 1),
                                )
                            nc.scalar.activation(
                                t8d[:, l, m, sl], ps[:], AFT.Tanh, scale=2.0
                            )
                    # u_i -> corri8[0, 0, l, :] = fp8(sum_w t_i)
                    for nb in range(NI_SH // 512):
                        sl = slice(nb * 512, (nb + 1) * 512)
                        ups = udpsum.tile([16, 512], F32, tag="udp")
                        for mm in (0, 2):
                            nc.tensor.matmul(
                                ups[:],
                                ones8[:],
                                t8i[:, l, mm : mm + 2, sl],
                                start=(mm == 0),
                                stop=(mm == 2),
                                perf_mode=DR,
                            )
                        nc.vector.tensor_copy(corri8[0:1, 0, l, sl], ups[0:1, :])
                    # u_d -> corrd8[0, 1, l, :] = fp8(sum_w t_d)
                    for ds in range(DSL):
                        dsl = slice(ds * 512, (ds + 1) * 512)
                        udps = udpsum.tile([16, 512], F32, tag="udp")
                        for mm in (0, 2):
                            nc.tensor.matmul(
                                udps[:],
                                ones8[:],
                                t8d[:, l, mm : mm + 2, dsl],
                                start=(mm == 0),
                                stop=(mm == 2),
                                perf_mode=DR,
                            )
                        nc.vector.tensor_copy(corrd8[0:1, 1, l, dsl], udps[0:1, :])

                # ---- Phase B: fp8 K-product ----
                for ic in range(I_CHUNKS):
                    isl = slice(ic * 128, (ic + 1) * 128)
                    for ds in range(DSL):
                        dsl = slice(ds * 512, (ds + 1) * 512)
                        kb = kpool.tile([128, 512], F32, tag="kb")
                        for l in range(DEPTH):
                            kps = kpsum.tile([128, 512], F32, tag="kps")
                            for mm in (0, 2):
                                nc.tensor.matmul(
                                    kps[:],
                                    t8i[:, l, mm : mm + 2, isl],
                                    t8d[:, l, mm : mm + 2, dsl],
                                    start=(mm == 0),
                                    stop=False,
                                    perf_mode=DR,
                                )
                            nc.tensor.matmul(
                                kps[:],
                                corri8[:, :, l, isl],
                                corrd8[:, :, l, dsl],
                                start=False,
                                stop=True,
                                perf_mode=DR,
                            )
                            if l == 0:
                                nc.vector.scalar_tensor_tensor(
                                    kb[:], kps[:], 512.0, alp[:, dsl],
                                    ADD, MULT,
                                )
                            elif l < DEPTH - 1:
                                nc.vector.scalar_tensor_tensor(
                                    kb[:], kps[:], 512.0, kb[:],
                                    ADD, MULT,
                                )
                            else:
                                part = scrpool.tile([128, 1], F32, tag="part")
                                nc.vector.scalar_tensor_tensor(
                                    kb[:], kps[:], 512.0, kb[:],
                                    ADD, MULT, accum_out=part[:, 0:1],
                                )
                                nc.gpsimd.tensor_add(
                                    y_acc[:, ic : ic + 1],
                                    y_acc[:, ic : ic + 1],
                                    part[:, 0:1],
                                )
                nc.vector.tensor_scalar_mul(y_acc[:], y_acc[:], 2.0 ** -35)
                nc.sync.dma_start(y_d.ap(), y_acc[:])

    nc.compile()
    return nc


def _get_nc():
    global _NC
    if _NC is None:
        _NC = _build_fp8()
    return _NC


def _make_in_maps(inp, data, gating, alphas):
    e4 = ml_dtypes.float8_e4m3fn
    ones8 = np.ones((128, 2, 16), dtype=e4)
    corrl8 = np.zeros((128, 2, 128), dtype=e4)
    corrl8[0, 0, :] = 64.0
    g0T = np.ascontiguousarray(gating[0].T)
    in_maps = []
    for r in range(R):
        inpT = np.ascontiguousarray(inp[r * NI_SH : (r + 1) * NI_SH].T)
        for c in range(C):
            dataT = np.ascontiguousarray(data[c * ND_SH : (c + 1) * ND_SH].T)
            al = np.ascontiguousarray(
                np.broadcast_to(alphas[c * ND_SH : (c + 1) * ND_SH], (128, ND_SH))
            )
            in_maps.append(
                {
                    "inpT": inpT,
                    "dataT": dataT,
                    "gating": gating,
                    "g0T": g0T,
                    "alphas_b": al,
                    "ones8": ones8,
                    "corrl8": corrl8,
                }
            )
    return in_maps


def kernel(inp, data, gating, alphas):
    inp = np.ascontiguousarray(np.asarray(inp, dtype=np.float32))
    data = np.ascontiguousarray(np.asarray(data, dtype=np.float32))
    gating = np.ascontiguousarray(np.asarray(gating, dtype=np.float32))
    alphas = np.ascontiguousarray(np.asarray(alphas, dtype=np.float32))

    nc = _get_nc()
    in_maps = _make_in_maps(inp, data, gating, alphas)
    res = run_bass_kernel_spmd(nc, in_maps, core_ids=list(range(R * C))).results

    y = np.empty(N_I, dtype=np.float32)
    for r in range(R):
        acc = res[r * C]["y"].T.reshape(NI_SH).copy()
        for c in range(1, C):
            acc += res[r * C + c]["y"].T.reshape(NI_SH)
        y[r * NI_SH : (r + 1) * NI_SH] = acc
    return y


# revision 7
# speedup vs baseline: 1.0245x; 1.0245x over previous
"""Trainium2 Bass kernel for the DLGN kernel-machine problem (fp8 DoubleRow).

Reference (fp32):
    ig = inp @ g0; dg = data @ g0
    K  = sig(4 ig) @ sig(4 dg).T
    for l in 1..3: ig @= g_l; dg @= g_l; K *= (sig(4 ig) @ sig(4 dg).T)/512
    out = K @ alphas                                  # [4096]

Design (8 cores, R=2 x C=4: inp rows split in 2, data rows in 4; each core
computes y_partial[r-block] over its data block; host sums C partials):
  - Gate chains are LINEAR in the preactivations: ig_l = inp @ (g0 g1..g_l).
    The kernel precomputes both weight-product chains on device (WP_l and
    its transpose chain V_l, plain f32r matmuls) and computes every layer's
    preactivation directly from inp/data -- no serial layer dependency, no
    PSUM->SBUF preactivation copies.
  - K-product runs in fp8 (float8e4) DoubleRow mode at 0.5 cyc/row (4x the
    f32r rate). Precision is recovered by tanh-centering: store
    t = fp8(tanh(2x)) (= 2 sig(4x) - 1), then
      sig-gram G = (T + u_i + u_d + 512) / 4,  T = t_i^T t_d (fp8 gram),
    with u_* = exact column sums of t (ones-matmuls). u_i enters as a
    per-partition scalar in the vector-engine multiply; u_d via one extra
    zero-padded DoubleRow pair (row0 = 64 * fp8(u_d/64)). The constant
    1/(4*2048^3) is applied once to y at the end (2^-35). Measured rel err
    ~1e-2 vs the 2e-2 gate (CPU model 9.8e-3).
  - ACT engine runs ONLY Tanh (scale=2, direct fp8 output -- bit-exact vs
    RNE cast); running K-product stays on DVE scalar_tensor_tensor with the
    alphas folded into layer 0 and accum_out row-sum on layer 3.
"""

import numpy as np
import ml_dtypes

import concourse.tile as tile
from concourse import bacc, mybir
from concourse.bass_utils import run_bass_kernel_spmd

WIDTH = 512
DEPTH = 4
DIM = 512
N_I = 4096
N_D = 8192
R, C = 2, 4
NI_SH = N_I // R  # 2048
ND_SH = N_D // C  # 2048
I_CHUNKS = NI_SH // 128  # 16
DSL = ND_SH // 512  # 4 d-blocks of 512
KC = DIM // 128  # 4

F32 = mybir.dt.float32
F32R = mybir.dt.float32r
FP8 = mybir.dt.float8e4
AFT = mybir.ActivationFunctionType
MULT = mybir.AluOpType.mult
ADD = mybir.AluOpType.add
DR = mybir.MatmulPerfMode.DoubleRow

_NC = None


def _build_fp8(repeat=1):
    nc = bacc.Bacc("TRN2", target_bir_lowering=False, debug=False, num_devices=8)

    inpT_d = nc.dram_tensor("inpT", [DIM, NI_SH], F32R, kind="ExternalInput")
    dataT_d = nc.dram_tensor("dataT", [DIM, ND_SH], F32R, kind="ExternalInput")
    gating_d = nc.dram_tensor("gating", [DEPTH, DIM, DIM], F32R, kind="ExternalInput")
    g0T_d = nc.dram_tensor("g0T", [DIM, DIM], F32R, kind="ExternalInput")
    alphas_d = nc.dram_tensor("alphas_b", [128, ND_SH], F32, kind="ExternalInput")
    ones_d = nc.dram_tensor("ones8", [128, 2, 16], FP8, kind="ExternalInput")
    corrl_d = nc.dram_tensor("corrl8", [128, 2, 128], FP8, kind="ExternalInput")
    y_d = nc.dram_tensor("y", [128, I_CHUNKS], F32, kind="ExternalOutput")

    from contextlib import nullcontext

    with tile.TileContext(nc) as tc:
        with (
            tc.tile_pool(name="wp", bufs=1) as wppool,
            tc.tile_pool(name="io", bufs=1) as iopool,
            tc.tile_pool(name="c8", bufs=1) as cpool,
            tc.For_i(0, repeat, 1) if repeat > 1 else nullcontext(),
        ):
            WP = wppool.tile([128, DEPTH, KC, DIM], F32R)
            inpT_sb = iopool.tile([128, KC, NI_SH], F32R, name="inpT_sb")
            dataT_sb = iopool.tile([128, KC, ND_SH], F32R, name="dataT_sb")
            alp = iopool.tile([128, ND_SH], F32, name="alp")
            ones8 = cpool.tile([128, 2, 16], FP8, name="ones8")
            corrl8 = cpool.tile([128, 2, 128], FP8, name="corrl8")
            nc.sync.dma_start(
                WP[:, 0], gating_d.ap()[0].rearrange("(k p) n -> p k n", p=128)
            )
            nc.sync.dma_start(ones8[:], ones_d.ap())
            nc.sync.dma_start(corrl8[:], corrl_d.ap())
            nc.sync.dma_start(alp[:], alphas_d.ap())
            inpT_r = inpT_d.ap().rearrange("(k p) n -> p k n", p=128)
            dataT_r = dataT_d.ap().rearrange("(k p) n -> p k n", p=128)
            for k in range(KC):
                nc.scalar.dma_start(inpT_sb[:, k], inpT_r[:, k])
                nc.scalar.dma_start(dataT_sb[:, k], dataT_r[:, k])

            # ---- Phase 0: weight-product chains ----
            with (
                tc.tile_pool(name="gv", bufs=1) as gvpool,
                tc.tile_pool(name="vv", bufs=2) as vpool,
                tc.tile_pool(name="p0", bufs=2, space="PSUM") as p0,
            ):
                g_all = gvpool.tile([128, DEPTH, KC, DIM], F32R)
                for l in range(1, DEPTH):
                    nc.sync.dma_start(
                        g_all[:, l],
                        gating_d.ap()[l].rearrange("(k p) n -> p k n", p=128),
                    )
                v_prev = vpool.tile([128, KC, DIM], F32R, tag="v")
                nc.sync.dma_start(
                    v_prev[:], g0T_d.ap().rearrange("(k p) n -> p k n", p=128)
                )
                for l in range(1, DEPTH):
                    # WP_l = WP_{l-1} @ g_l  (lhsT = V_{l-1})
                    for ma in range(KC):
                        ps = p0.tile([128, 512], F32, tag="p0")
                        for k in range(KC):
                            nc.tensor.matmul(
                                ps[:],
                                v_prev[:, k, ma * 128 : (ma + 1) * 128],
                                g_all[:, l, k, :],
                                start=(k == 0),
                                stop=(k == KC - 1),
                            )
                        nc.vector.tensor_copy(WP[:, l, ma, :], ps[:])
                    if l < DEPTH - 1:
                        v_next = vpool.tile([128, KC, DIM], F32R, tag="v", name=f"v{l}")
                        for mw in range(KC):
                            ps = p0.tile([128, 512], F32, tag="p0")
                            for k in range(KC):
                                nc.tensor.matmul(
                                    ps[:],
                                    g_all[:, l, k, mw * 128 : (mw + 1) * 128],
                                    v_prev[:, k, :],
                                    start=(k == 0),
                                    stop=(k == KC - 1),
                                )
                            nc.vector.tensor_copy(v_next[:, mw, :], ps[:])
                        v_prev = v_next

            # ---- Phase A: all gates (i and d side), u-sums ----
            with (
                tc.tile_pool(name="sig", bufs=1) as sigpool,
                tc.tile_pool(name="kb", bufs=3) as kpool,
                tc.tile_pool(name="scr", bufs=4) as scrpool,
                tc.tile_pool(name="gps", bufs=3, space="PSUM") as gpsum,
                tc.tile_pool(name="udp", bufs=2, space="PSUM") as udpsum,
                tc.tile_pool(name="kps", bufs=3, space="PSUM") as kpsum,
            ):
                t8i = sigpool.tile([128, DEPTH, KC, NI_SH], FP8)
                t8d = sigpool.tile([128, DEPTH, KC, ND_SH], FP8)
                corri8 = sigpool.tile([128, 2, DEPTH, NI_SH], FP8)
                corrd8 = sigpool.tile([128, 2, DEPTH, ND_SH], FP8)
                y_acc = sigpool.tile([128, I_CHUNKS], F32)
                nc.gpsimd.memset(corri8[:], 0.0)
                nc.gpsimd.memset(corrd8[:], 0.0)
                for l in range(DEPTH):
                    nc.gpsimd.memset(corri8[0:1, 1, l, :], 1.0)
                    nc.gpsimd.memset(corrd8[0:1, 0, l, :], 1.0)
                nc.gpsimd.memset(y_acc[:], 0.0)

                for l in range(DEPTH):
                    for m in range(KC):
                        for nb in range(NI_SH // 512):
                            sl = slice(nb * 512, (nb + 1) * 512)
                            ps = gpsum.tile([128, 512], F32, tag="gps")
                            for k in range(KC):
                                nc.tensor.matmul(
                                    ps[:],
                                    WP[:, l, k, m * 128 : (m + 1) * 128],
                                    inpT_sb[:, k, sl],
                                    start=(k == 0),
                                    stop=(k == KC - 1),
                                )
                            nc.scalar.activation(
                                t8i[:, l, m, sl], ps[:], AFT.Tanh, scale=2.0
                            )
                    for m in range(KC):
                        for nb in range(ND_SH // 512):
                            sl = slice(nb * 512, (nb + 1) * 512)
                            ps = gpsum.tile([128, 512], F32, tag="gps")
                            for k in range(KC):
                                nc.tensor.matmul(
                                    ps[:],
                                    WP[:, l, k, m * 128 : (m + 1) * 128],
                                    dataT_sb[:, k, sl],
                                    start=(k == 0),
                                    stop=(k == KC - 1),
                                )
                            nc.scalar.activation(
                                t8d[:, l, m, sl], ps[:], AFT.Tanh, scale=2.0
                            )
                    # u_i -> corri8[0, 0, l, :] = fp8(sum_w t_i)
                    for nb in range(NI_SH // 512):
                        sl = slice(nb * 512, (nb + 1) * 512)
                        ups = udpsum.tile([16, 512], F32, tag="udp")
                        for mm in (0, 2):
                            nc.tensor.matmul(
                                ups[:],
                                ones8[:],
                                t8i[:, l, mm : mm + 2, sl],
                                start=(mm == 0),
                                stop=(mm == 2),
                                perf_mode=DR,
                            )
                        nc.vector.tensor_copy(corri8[0:16, 0, l, sl], ups[:, :])
                    # u_d -> corrd8[0, 1, l, :] = fp8(sum_w t_d)
                    for ds in range(DSL):
                        dsl = slice(ds * 512, (ds + 1) * 512)
                        udps = udpsum.tile([16, 512], F32, tag="udp")
                        for mm in (0, 2):
                            nc.tensor.matmul(
                                udps[:],
                                ones8[:],
                                t8d[:, l, mm : mm + 2, dsl],
                                start=(mm == 0),
                                stop=(mm == 2),
                                perf_mode=DR,
                            )
                        nc.vector.tensor_copy(corrd8[0:16, 1, l, dsl], udps[:, :])

                # ---- Phase B: fp8 K-product ----
                for ic in range(I_CHUNKS):
                    isl = slice(ic * 128, (ic + 1) * 128)
                    for ds in range(DSL):
                        dsl = slice(ds * 512, (ds + 1) * 512)
                        kb = kpool.tile([128, 512], F32, tag="kb")
                        for l in range(DEPTH):
                            kps = kpsum.tile([128, 512], F32, tag="kps")
                            for mm in (0, 2):
                                nc.tensor.matmul(
                                    kps[:],
                                    t8i[:, l, mm : mm + 2, isl],
                                    t8d[:, l, mm : mm + 2, dsl],
                                    start=(mm == 0),
                                    stop=False,
                                    perf_mode=DR,
                                )
                            nc.tensor.matmul(
                                kps[:],
                                corri8[:, :, l, isl],
                                corrd8[:, :, l, dsl],
                                start=False,
                                stop=True,
                                perf_mode=DR,
                            )
                            if l == 0:
                                nc.vector.scalar_tensor_tensor(
                                    kb[:], kps[:], 512.0, alp[:, dsl],
                                    ADD, MULT,
                                )
                            elif l < DEPTH - 1:
                                nc.vector.scalar_tensor_tensor(
                                    kb[:], kps[:], 512.0, kb[:],
                                    ADD, MULT,
                                )
                            else:
                                part = scrpool.tile([128, 1], F32, tag="part")
                                nc.vector.scalar_tensor_tensor(
                                    kb[:], kps[:], 512.0, kb[:],
                                    ADD, MULT, accum_out=part[:, 0:1],
                                )
                                nc.gpsimd.tensor_add(
                                    y_acc[:, ic : ic + 1],
                                    y_acc[:, ic : ic + 1],
                                    part[:, 0:1],
                                )
                nc.vector.tensor_scalar_mul(y_acc[:], y_acc[:], 2.0 ** -35)
                nc.sync.dma_start(y_d.ap(), y_acc[:])

    nc.compile()
    return nc


def _get_nc():
    global _NC
    if _NC is None:
        _NC = _build_fp8()
    return _NC


def _make_in_maps(inp, data, gating, alphas):
    e4 = ml_dtypes.float8_e4m3fn
    ones8 = np.zeros((128, 2, 16), dtype=e4)
    ones8[:, :, 0] = 1.0
    corrl8 = np.zeros((128, 2, 128), dtype=e4)
    corrl8[0, 0, :] = 64.0
    g0T = np.ascontiguousarray(gating[0].T)
    in_maps = []
    for r in range(R):
        inpT = np.ascontiguousarray(inp[r * NI_SH : (r + 1) * NI_SH].T)
        for c in range(C):
            dataT = np.ascontiguousarray(data[c * ND_SH : (c + 1) * ND_SH].T)
            al = np.ascontiguousarray(
                np.broadcast_to(alphas[c * ND_SH : (c + 1) * ND_SH], (128, ND_SH))
            )
            in_maps.append(
                {
                    "inpT": inpT,
                    "dataT": dataT,
                    "gating": gating,
                    "g0T": g0T,
                    "alphas_b": al,
                    "ones8": ones8,
                    "corrl8": corrl8,
                }
            )
    return in_maps


def kernel(inp, data, gating, alphas):
    inp = np.ascontiguousarray(np.asarray(inp, dtype=np.float32))
    data = np.ascontiguousarray(np.asarray(data, dtype=np.float32))
    gating = np.ascontiguousarray(np.asarray(gating, dtype=np.float32))
    alphas = np.ascontiguousarray(np.asarray(alphas, dtype=np.float32))

    nc = _get_nc()
    in_maps = _make_in_maps(inp, data, gating, alphas)
    res = run_bass_kernel_spmd(nc, in_maps, core_ids=list(range(R * C))).results

    y = np.empty(N_I, dtype=np.float32)
    for r in range(R):
        acc = res[r * C]["y"].T.reshape(NI_SH).copy()
        for c in range(1, C):
            acc += res[r * C + c]["y"].T.reshape(NI_SH)
        y[r * NI_SH : (r + 1) * NI_SH] = acc
    return y


# revision 9
# speedup vs baseline: 1.0920x; 1.0660x over previous
"""Trainium2 Bass kernel for the DLGN kernel-machine problem (fp8 DoubleRow).

Reference (fp32):
    ig = inp @ g0; dg = data @ g0
    K  = sig(4 ig) @ sig(4 dg).T
    for l in 1..3: ig @= g_l; dg @= g_l; K *= (sig(4 ig) @ sig(4 dg).T)/512
    out = K @ alphas                                  # [4096]

Design (8 cores, R=2 x C=4: inp rows split in 2, data rows in 4; each core
computes y_partial[r-block] over its data block; host sums C partials):
  - Gate chains are LINEAR in the preactivations: ig_l = inp @ (g0 g1..g_l).
    The kernel precomputes both weight-product chains on device (WP_l and
    its transpose chain V_l, plain f32r matmuls) and computes every layer's
    preactivation directly from inp/data -- no serial layer dependency, no
    PSUM->SBUF preactivation copies.
  - K-product runs in fp8 (float8e4) DoubleRow mode at 0.5 cyc/row (4x the
    f32r rate). Precision is recovered by tanh-centering: store
    t = fp8(tanh(2x)) (= 2 sig(4x) - 1), then
      sig-gram G = (T + u_i + u_d + 512) / 4,  T = t_i^T t_d (fp8 gram),
    with u_* = exact column sums of t (ones-matmuls). u_i enters as a
    per-partition scalar in the vector-engine multiply; u_d via one extra
    zero-padded DoubleRow pair (row0 = 64 * fp8(u_d/64)). The constant
    1/(4*2048^3) is applied once to y at the end (2^-35). Measured rel err
    ~1e-2 vs the 2e-2 gate (CPU model 9.8e-3).
  - ACT engine runs ONLY Tanh (scale=2, direct fp8 output -- bit-exact vs
    RNE cast); running K-product stays on DVE scalar_tensor_tensor with the
    alphas folded into layer 0 and accum_out row-sum on layer 3.
"""

import numpy as np
import ml_dtypes

import concourse.tile as tile
from concourse import bacc, mybir
from concourse.bass_utils import run_bass_kernel_spmd

WIDTH = 512
DEPTH = 4
DIM = 512
N_I = 4096
N_D = 8192
R, C = 2, 4
NI_SH = N_I // R  # 2048
ND_SH = N_D // C  # 2048
I_CHUNKS = NI_SH // 128  # 16
DSL = ND_SH // 512  # 4 d-blocks of 512
KC = DIM // 128  # 4

F32 = mybir.dt.float32
F32R = mybir.dt.float32r
FP8 = mybir.dt.float8e4
AFT = mybir.ActivationFunctionType
MULT = mybir.AluOpType.mult
ADD = mybir.AluOpType.add
DR = mybir.MatmulPerfMode.DoubleRow

_NC = None


def _build_fp8(repeat=1):
    nc = bacc.Bacc("TRN2", target_bir_lowering=False, debug=False, num_devices=8)

    inpT_d = nc.dram_tensor("inpT", [DIM, NI_SH], F32R, kind="ExternalInput")
    dataT_d = nc.dram_tensor("dataT", [DIM, ND_SH], F32R, kind="ExternalInput")
    gating_d = nc.dram_tensor("gating", [DEPTH, DIM, DIM], F32R, kind="ExternalInput")
    g0T_d = nc.dram_tensor("g0T", [DIM, DIM], F32R, kind="ExternalInput")
    alphas_d = nc.dram_tensor("alphas_b", [128, ND_SH], F32, kind="ExternalInput")
    ones_d = nc.dram_tensor("ones8", [128, 2, 16], FP8, kind="ExternalInput")
    corrl_d = nc.dram_tensor("corrl8", [128, 2, 128], FP8, kind="ExternalInput")
    y_d = nc.dram_tensor("y", [128, I_CHUNKS], F32, kind="ExternalOutput")

    from contextlib import nullcontext

    with tile.TileContext(nc) as tc:
        with (
            tc.tile_pool(name="wp", bufs=1) as wppool,
            tc.tile_pool(name="io", bufs=1) as iopool,
            tc.tile_pool(name="c8", bufs=1) as cpool,
            tc.For_i(0, repeat, 1) if repeat > 1 else nullcontext(),
        ):
            WP = wppool.tile([128, DEPTH, KC, DIM], F32R)
            inpT_sb = iopool.tile([128, KC, NI_SH], F32R, name="inpT_sb")
            dataT_sb = iopool.tile([128, KC, ND_SH], F32R, name="dataT_sb")
            alp = iopool.tile([128, ND_SH], F32, name="alp")
            ones8 = cpool.tile([128, 2, 16], FP8, name="ones8")
            corrl8 = cpool.tile([128, 2, 128], FP8, name="corrl8")
            nc.sync.dma_start(
                WP[:, 0], gating_d.ap()[0].rearrange("(k p) n -> p k n", p=128)
            )
            nc.sync.dma_start(ones8[:], ones_d.ap())
            nc.sync.dma_start(corrl8[:], corrl_d.ap())
            nc.sync.dma_start(alp[:], alphas_d.ap())
            inpT_r = inpT_d.ap().rearrange("(k p) n -> p k n", p=128)
            dataT_r = dataT_d.ap().rearrange("(k p) n -> p k n", p=128)
            for k in range(KC):
                nc.scalar.dma_start(inpT_sb[:, k], inpT_r[:, k])
                nc.scalar.dma_start(dataT_sb[:, k], dataT_r[:, k])

            # ---- Phase 0: weight-product chains ----
            with (
                tc.tile_pool(name="gv", bufs=1) as gvpool,
                tc.tile_pool(name="vv", bufs=2) as vpool,
                tc.tile_pool(name="p0", bufs=2, space="PSUM") as p0,
            ):
                g_all = gvpool.tile([128, DEPTH, KC, DIM], F32R)
                for l in range(1, DEPTH):
                    nc.sync.dma_start(
                        g_all[:, l],
                        gating_d.ap()[l].rearrange("(k p) n -> p k n", p=128),
                    )
                v_prev = vpool.tile([128, KC, DIM], F32R, tag="v")
                nc.sync.dma_start(
                    v_prev[:], g0T_d.ap().rearrange("(k p) n -> p k n", p=128)
                )
                for l in range(1, DEPTH):
                    # WP_l = WP_{l-1} @ g_l  (lhsT = V_{l-1})
                    for ma in range(KC):
                        ps = p0.tile([128, 512], F32, tag="p0")
                        for k in range(KC):
                            nc.tensor.matmul(
                                ps[:],
                                v_prev[:, k, ma * 128 : (ma + 1) * 128],
                                g_all[:, l, k, :],
                                start=(k == 0),
                                stop=(k == KC - 1),
                            )
                        nc.vector.tensor_copy(WP[:, l, ma, :], ps[:])
                    if l < DEPTH - 1:
                        v_next = vpool.tile([128, KC, DIM], F32R, tag="v", name=f"v{l}")
                        for mw in range(KC):
                            ps = p0.tile([128, 512], F32, tag="p0")
                            for k in range(KC):
                                nc.tensor.matmul(
                                    ps[:],
                                    g_all[:, l, k, mw * 128 : (mw + 1) * 128],
                                    v_prev[:, k, :],
                                    start=(k == 0),
                                    stop=(k == KC - 1),
                                )
                            nc.vector.tensor_copy(v_next[:, mw, :], ps[:])
                        v_prev = v_next

            # ---- Phase A: all gates (i and d side), u-sums ----
            with (
                tc.tile_pool(name="sig", bufs=1) as sigpool,
                tc.tile_pool(name="kb", bufs=3) as kpool,
                tc.tile_pool(name="scr", bufs=4) as scrpool,
                tc.tile_pool(name="gps", bufs=3, space="PSUM") as gpsum,
                tc.tile_pool(name="udp", bufs=2, space="PSUM") as udpsum,
                tc.tile_pool(name="kps", bufs=3, space="PSUM") as kpsum,
            ):
                t8i = sigpool.tile([128, DEPTH, KC, NI_SH], FP8)
                t8d = sigpool.tile([128, DEPTH, KC, ND_SH], FP8)
                corri8 = sigpool.tile([128, 2, DEPTH, NI_SH], FP8)
                corrd8 = sigpool.tile([128, 2, DEPTH, ND_SH], FP8)
                y_acc = sigpool.tile([128, I_CHUNKS], F32)
                nc.gpsimd.memset(corri8[:], 0.0)
                nc.gpsimd.memset(corrd8[:], 0.0)
                for l in range(DEPTH):
                    nc.gpsimd.memset(corri8[0:1, 1, l, :], 1.0)
                    nc.gpsimd.memset(corrd8[0:1, 0, l, :], 1.0)
                nc.gpsimd.memset(y_acc[:], 0.0)

                for l in range(DEPTH):
                    for m in range(KC):
                        for nb in range(NI_SH // 512):
                            sl = slice(nb * 512, (nb + 1) * 512)
                            ps = gpsum.tile([128, 512], F32, tag="gps")
                            for k in range(KC):
                                nc.tensor.matmul(
                                    ps[:],
                                    WP[:, l, k, m * 128 : (m + 1) * 128],
                                    inpT_sb[:, k, sl],
                                    start=(k == 0),
                                    stop=(k == KC - 1),
                                )
                            nc.scalar.activation(
                                t8i[:, l, m, sl], ps[:], AFT.Tanh, scale=2.0
                            )
                    for m in range(KC):
                        for nb in range(ND_SH // 512):
                            sl = slice(nb * 512, (nb + 1) * 512)
                            ps = gpsum.tile([128, 512], F32, tag="gps")
                            for k in range(KC):
                                nc.tensor.matmul(
                                    ps[:],
                                    WP[:, l, k, m * 128 : (m + 1) * 128],
                                    dataT_sb[:, k, sl],
                                    start=(k == 0),
                                    stop=(k == KC - 1),
                                )
                            nc.scalar.activation(
                                t8d[:, l, m, sl], ps[:], AFT.Tanh, scale=2.0
                            )
                    # u_i -> corri8[0, 0, l, :] = fp8(sum_w t_i)
                    for nb in range(NI_SH // 512):
                        sl = slice(nb * 512, (nb + 1) * 512)
                        ups = udpsum.tile([16, 512], F32, tag="udp")
                        for mm in (0, 2):
                            nc.tensor.matmul(
                                ups[:],
                                ones8[:],
                                t8i[:, l, mm : mm + 2, sl],
                                start=(mm == 0),
                                stop=(mm == 2),
                                perf_mode=DR,
                            )
                        nc.vector.tensor_copy(corri8[0:16, 0, l, sl], ups[:, :])
                    # u_d -> corrd8[0, 1, l, :] = fp8(sum_w t_d)
                    for ds in range(DSL):
                        dsl = slice(ds * 512, (ds + 1) * 512)
                        udps = udpsum.tile([16, 512], F32, tag="udp")
                        for mm in (0, 2):
                            nc.tensor.matmul(
                                udps[:],
                                ones8[:],
                                t8d[:, l, mm : mm + 2, dsl],
                                start=(mm == 0),
                                stop=(mm == 2),
                                perf_mode=DR,
                            )
                        nc.vector.tensor_copy(corrd8[0:16, 1, l, dsl], udps[:, :])

                # ---- Phase B: fp8 K-product ----
                for ic in range(I_CHUNKS):
                    isl = slice(ic * 128, (ic + 1) * 128)
                    for ds in range(DSL):
                        dsl = slice(ds * 512, (ds + 1) * 512)
                        kb = kpool.tile([128, 512], F32, tag="kb")
                        for l in range(DEPTH):
                            kps = kpsum.tile([128, 512], F32, tag="kps")
                            for mm in (0, 2):
                                nc.tensor.matmul(
                                    kps[:],
                                    t8i[:, l, mm : mm + 2, isl],
                                    t8d[:, l, mm : mm + 2, dsl],
                                    start=(mm == 0),
                                    stop=False,
                                    perf_mode=DR,
                                )
                            nc.tensor.matmul(
                                kps[:],
                                corri8[:, :, l, isl],
                                corrd8[:, :, l, dsl],
                                start=False,
                                stop=True,
                                perf_mode=DR,
                            )
                            if l == 0:
                                nc.vector.scalar_tensor_tensor(
                                    kb[:], kps[:], 512.0, alp[:, dsl],
                                    ADD, MULT,
                                )
                            elif l < DEPTH - 1:
                                nc.vector.scalar_tensor_tensor(
                                    kb[:], kps[:], 512.0, kb[:],
                                    ADD, MULT,
                                )
                            else:
                                part = scrpool.tile([128, 1], F32, tag="part")
                                nc.vector.scalar_tensor_tensor(
                                    kb[:], kps[:], 512.0, kb[:],
                                    ADD, MULT, accum_out=part[:, 0:1],
                                )
                                nc.gpsimd.tensor_add(
                                    y_acc[:, ic : ic + 1],
                                    y_acc[:, ic : ic + 1],
                                    part[:, 0:1],
                                )
                nc.vector.tensor_scalar_mul(y_acc[:], y_acc[:], 2.0 ** -35)
                nc.sync.dma_start(y_d.ap(), y_acc[:])

    nc.compile()
    return nc


def _get_nc():
    global _NC
    if _NC is None:
        _NC = _build_fp8()
    return _NC


def _make_in_maps(inp, data, gating, alphas):
    e4 = ml_dtypes.float8_e4m3fn
    ones8 = np.zeros((128, 2, 16), dtype=e4)
    ones8[:, :, 0] = 1.0
    corrl8 = np.zeros((128, 2, 128), dtype=e4)
    corrl8[0, 0, :] = 64.0
    g0T = np.ascontiguousarray(gating[0].T)
    in_maps = []
    for r in range(R):
        inpT = np.ascontiguousarray(inp[r * NI_SH : (r + 1) * NI_SH].T)
        for c in range(C):
            dataT = np.ascontiguousarray(data[c * ND_SH : (c + 1) * ND_SH].T)
            al = np.ascontiguousarray(
                np.broadcast_to(alphas[c * ND_SH : (c + 1) * ND_SH], (128, ND_SH))
            )
            in_maps.append(
                {
                    "inpT": inpT,
                    "dataT": dataT,
                    "gating": gating,
                    "g0T": g0T,
                    "alphas_b": al,
                    "ones8": ones8,
                    "corrl8": corrl8,
                }
            )
    return in_maps


def kernel(inp, data, gating, alphas):
    inp = np.ascontiguousarray(np.asarray(inp, dtype=np.float32))
    data = np.ascontiguousarray(np.asarray(data, dtype=np.float32))
    gating = np.ascontiguousarray(np.asarray(gating, dtype=np.float32))
    alphas = np.ascontiguousarray(np.asarray(alphas, dtype=np.float32))

    nc = _get_nc()
    in_maps = _make_in_maps(inp, data, gating, alphas)
    res = run_bass_kernel_spmd(nc, in_maps, core_ids=list(range(R * C))).results

    y = np.empty(N_I, dtype=np.float32)
    for r in range(R):
        acc = res[r * C]["y"].T.reshape(NI_SH).copy()
        for c in range(1, C):
            acc += res[r * C + c]["y"].T.reshape(NI_SH)
        y[r * NI_SH : (r + 1) * NI_SH] = acc
    return y
